# revision 1
# baseline (speedup 1.0000x reference)
"""Inverse 3D DWT (db4, 2 levels) Trainium2 Bass kernel.

Sharding: 8 cores = 4 batch samples x 2 halves of the output H axis.
Each core reconstructs its (256, 128, 256) output slab independently
(H-halo of +/-2 on the level-1 inputs, periodic), as six banded-matmul
stages (filter D, H, W per level) on the PE array in fp32r.

Self-contained: hardcodes shapes for inputs
  yl (4,1,64,64,64), yh1 (4,7,128,128,128), yh2 (4,7,64,64,64), g0/g1 (8,).
"""
import sys
import numpy as np

sys.path.insert(0, "/opt/trn_rl_repo")

_CACHE = {}


def _band_cyclic(w, n):
    M = np.zeros((n, 2 * n), np.float32)
    for j in range(2 * n):
        k0 = (j + 3) % 2
        base = (j + 3) // 2
        for s in range(4):
            M[(base - s) % n, j] += w[k0 + 2 * s]
    return M


def _band_slab(w, K, nout):
    M = np.zeros((K, nout), np.float32)
    for p in range(K):
        for j in range(nout):
            k = j + 7 - 2 * p
            if 0 <= k < 8:
                M[p, j] = w[k]
    return M


def _build_nc():
    import concourse.bass as bass  # noqa: F401
    import concourse.tile as tile
    import concourse.mybir as mybir
    from concourse import bacc

    f32 = mybir.dt.float32
    f32r = mybir.dt.float32r

    nc = bacc.Bacc("TRN2", debug=False, num_devices=8)

    yl_s = nc.dram_tensor("yl_s", [64, 64, 64], f32r, kind="ExternalInput")
    yh2_s = nc.dram_tensor("yh2_s", [7, 64, 64, 64], f32r, kind="ExternalInput")
    yh1_s = nc.dram_tensor("yh1_s", [7, 128, 68, 128], f32r, kind="ExternalInput")
    ident = nc.dram_tensor("ident", [128, 128], f32r, kind="ExternalInput")
    mc64 = nc.dram_tensor("mc64", [2, 64, 128], f32r, kind="ExternalInput")
    ms2 = nc.dram_tensor("ms2", [2, 64, 68], f32r, kind="ExternalInput")
    mc128 = nc.dram_tensor("mc128", [2, 128, 256], f32r, kind="ExternalInput")
    mt2 = nc.dram_tensor("mt2", [2, 68, 128], f32r, kind="ExternalInput")
    out = nc.dram_tensor("out", [256, 32768], f32, kind="ExternalOutput")

    with tile.TileContext(nc) as tc:
        with (
            tc.tile_pool(name="const", bufs=1) as cp,
            tc.tile_pool(name="ld", bufs=6) as ld,
            tc.tile_pool(name="xt", bufs=4) as xtp,
            tc.tile_pool(name="st", bufs=6) as stp,
            tc.tile_pool(name="ps", bufs=4, space="PSUM") as ps,
            tc.tile_pool(name="pst", bufs=2, space="PSUM") as pst,
            tc.tile_pool(name="dram", bufs=1, space="DRAM") as dp,
        ):
            # constants to SBUF
            idt = cp.tile([128, 128], f32r, tag="idt")
            nc.sync.dma_start(idt[:], ident[:])
            c64, cs2, c128, ct2 = [], [], [], []
            for a in range(2):
                t = cp.tile([64, 128], f32r, tag=f"c64_{a}")
                nc.sync.dma_start(t[:], mc64[a])
                c64.append(t)
                t = cp.tile([64, 68], f32r, tag=f"cs2_{a}")
                nc.sync.dma_start(t[:], ms2[a])
                cs2.append(t)
                t = cp.tile([128, 256], f32r, tag=f"c128_{a}")
                nc.sync.dma_start(t[:], mc128[a])
                c128.append(t)
                t = cp.tile([68, 128], f32r, tag=f"ct2_{a}")
                nc.sync.dma_start(t[:], mt2[a])
                ct2.append(t)

            # DRAM scratch
            s1o = dp.tile([4, 128, 4096], f32r, tag="s1o")
            s2o = dp.tile([2, 8704, 64], f32r, tag="s2o")
            llw = dp.tile([128, 68, 128], f32r, tag="llw")
            lld = dp.tile([128, 8704], f32r, tag="lld")
            t1o = dp.tile([4, 256, 8704], f32r, tag="t1o")
            t2o = dp.tile([2, 32768, 128], f32r, tag="t2o")

            ys2d = [yl_s.ap().rearrange("d h w -> d (h w)")] + [
                yh2_s[i].rearrange("d h w -> d (h w)") for i in range(7)
            ]

            # ---- S1: level-2 filter D.  K=64 -> 128 outs, lines=(h,w)=4096
            for p in range(4):
                lo2, hi2 = ys2d[2 * p], ys2d[2 * p + 1]
                for j in range(8):
                    tl = ld.tile([64, 512], f32r, tag="s1l")
                    nc.sync.dma_start(tl[:], lo2[:, j * 512:(j + 1) * 512])
                    th = ld.tile([64, 512], f32r, tag="s1h")
                    nc.sync.dma_start(th[:], hi2[:, j * 512:(j + 1) * 512])
                    acc = ps.tile([128, 512], f32, tag="acc")
                    nc.tensor.matmul(acc[:], c64[0][:], tl[:], start=True, stop=False)
                    nc.tensor.matmul(acc[:], c64[1][:], th[:], start=False, stop=True)
                    so = stp.tile([128, 512], f32r, tag="s1o")
                    nc.any.tensor_copy(so[:], acc[:])
                    nc.sync.dma_start(s1o[p, :, j * 512:(j + 1) * 512], so[:])

            # ---- S2: level-2 filter H -> 68-window.  K=64, lines=(d,w)=8192
            for u in range(2):
                lo3 = s1o[2 * u].rearrange("d (h w) -> h d w", h=64)
                hi3 = s1o[2 * u + 1].rearrange("d (h w) -> h d w", h=64)
                d2 = s2o[u].rearrange("(h d) w -> h (d w)", h=68)
                for j in range(16):
                    tl = ld.tile([64, 8, 64], f32r, tag="s2l")
                    nc.sync.dma_start(tl[:], lo3[:, j * 8:(j + 1) * 8, :])
                    th = ld.tile([64, 8, 64], f32r, tag="s2h")
                    nc.sync.dma_start(th[:], hi3[:, j * 8:(j + 1) * 8, :])
                    acc = ps.tile([128, 512], f32, tag="acc")
                    nc.tensor.matmul(acc[:68, :], cs2[0][:], tl[:].rearrange("p a b -> p (a b)"), start=True, stop=False)
                    nc.tensor.matmul(acc[:68, :], cs2[1][:], th[:].rearrange("p a b -> p (a b)"), start=False, stop=True)
                    so = stp.tile([68, 512], f32r, tag="s2o")
                    nc.any.tensor_copy(so[:], acc[:68, :])
                    nc.sync.dma_start(d2[:, j * 512:(j + 1) * 512], so[:])

            # ---- S3: level-2 filter W (cyclic 64).  PE-transpose then matmul.
            for g in range(17):
                xtsb = []
                for k in range(2):
                    v = s2o[k]  # lines=(h,d) rows
                    xt = pst.tile([64, 512], f32r, tag="xtp")
                    for q in range(4):
                        r0 = (g * 4 + q) * 128
                        tl = ld.tile([128, 64], f32r, tag="s3l")
                        nc.sync.dma_start(tl[:], v[r0:r0 + 128, :])
                        nc.tensor.transpose(xt[:, q * 128:(q + 1) * 128], tl[:], idt[:])
                    xts = xtp.tile([64, 512], f32r, tag="xts3")
                    nc.any.tensor_copy(xts[:], xt[:])
                    xtsb.append(xts)
                acc = ps.tile([128, 512], f32, tag="acc")
                nc.tensor.matmul(acc[:], c64[0][:], xtsb[0][:], start=True, stop=False)
                nc.tensor.matmul(acc[:], c64[1][:], xtsb[1][:], start=False, stop=True)
                so = stp.tile([128, 4, 128], f32r, tag="s3o")
                nc.any.tensor_copy(so[:].rearrange("p a b -> p (a b)"), acc[:])
                nc.sync.dma_start(llw[:, g * 4:(g + 1) * 4, :], so[:])

            # ---- transpose ll (W,H,D) -> (D,H,W)
            lld3 = lld.rearrange("d (h w) -> d h w", h=68)
            for h in range(68):
                tl = ld.tile([128, 128], f32r, tag="llt")
                nc.sync.dma_start(tl[:], llw[:, h, :])
                tp = pst.tile([128, 128], f32r, tag="xtp")
                nc.tensor.transpose(tp[:], tl[:], idt[:])
                so = stp.tile([128, 128], f32r, tag="llto")
                nc.any.tensor_copy(so[:], tp[:])
                nc.sync.dma_start(lld3[:, h, :], so[:])

            # ---- T1: level-1 filter D (cyclic 128). K=128 -> 256 outs, lines=8704
            ys1 = [lld[:]] + [yh1_s[i].rearrange("d h w -> d (h w)") for i in range(7)]
            for p in range(4):
                lo2, hi2 = ys1[2 * p], ys1[2 * p + 1]
                for j in range(17):
                    tl = ld.tile([128, 512], f32r, tag="t1l")
                    nc.sync.dma_start(tl[:], lo2[:, j * 512:(j + 1) * 512])
                    th = ld.tile([128, 512], f32r, tag="t1h")
                    nc.sync.dma_start(th[:], hi2[:, j * 512:(j + 1) * 512])
                    for oc in range(2):
                        acc = ps.tile([128, 512], f32, tag="acc")
                        nc.tensor.matmul(acc[:], c128[0][:][:, oc * 128:(oc + 1) * 128], tl[:], start=True, stop=False)
                        nc.tensor.matmul(acc[:], c128[1][:][:, oc * 128:(oc + 1) * 128], th[:], start=False, stop=True)
                        so = stp.tile([128, 512], f32r, tag="t1o")
                        nc.any.tensor_copy(so[:], acc[:])
                        nc.sync.dma_start(t1o[p, oc * 128:(oc + 1) * 128, j * 512:(j + 1) * 512], so[:])

            # ---- T2: level-1 filter H (slab 68->128). K=68, lines=(d,w)=32768
            for u in range(2):
                lo3 = t1o[2 * u].rearrange("d (h w) -> h d w", h=68)
                hi3 = t1o[2 * u + 1].rearrange("d (h w) -> h d w", h=68)
                d2 = t2o[u].rearrange("(h d) w -> h (d w)", h=128)
                for j in range(64):
                    tl = ld.tile([68, 4, 128], f32r, tag="t2l")
                    nc.sync.dma_start(tl[:], lo3[:, j * 4:(j + 1) * 4, :])
                    th = ld.tile([68, 4, 128], f32r, tag="t2h")
                    nc.sync.dma_start(th[:], hi3[:, j * 4:(j + 1) * 4, :])
                    acc = ps.tile([128, 512], f32, tag="acc")
                    nc.tensor.matmul(acc[:], ct2[0][:], tl[:].rearrange("p a b -> p (a b)"), start=True, stop=False)
                    nc.tensor.matmul(acc[:], ct2[1][:], th[:].rearrange("p a b -> p (a b)"), start=False, stop=True)
                    so = stp.tile([128, 512], f32r, tag="t2s")
                    nc.any.tensor_copy(so[:], acc[:])
                    nc.sync.dma_start(d2[:, j * 512:(j + 1) * 512], so[:])

            # ---- T3: level-1 filter W (cyclic 128). PE-transpose + matmul.
            for g in range(64):
                xtsb = []
                for k in range(2):
                    v = t2o[k]
                    xt = pst.tile([128, 512], f32r, tag="xtp")
                    for q in range(4):
                        r0 = (g * 4 + q) * 128
                        tl = ld.tile([128, 128], f32r, tag="t3l")
                        nc.sync.dma_start(tl[:], v[r0:r0 + 128, :])
                        nc.tensor.transpose(xt[:, q * 128:(q + 1) * 128], tl[:], idt[:])
                    xts = xtp.tile([128, 512], f32r, tag="xtsT")
                    nc.any.tensor_copy(xts[:], xt[:])
                    xtsb.append(xts)
                for oc in range(2):
                    acc = ps.tile([128, 512], f32, tag="acc")
                    nc.tensor.matmul(acc[:], c128[0][:][:, oc * 128:(oc + 1) * 128], xtsb[0][:], start=True, stop=False)
                    nc.tensor.matmul(acc[:], c128[1][:][:, oc * 128:(oc + 1) * 128], xtsb[1][:], start=False, stop=True)
                    so = stp.tile([128, 512], f32, tag="t3o")
                    nc.any.tensor_copy(so[:], acc[:])
                    nc.sync.dma_start(out[oc * 128:(oc + 1) * 128, g * 512:(g + 1) * 512], so[:])

    nc.finalize()
    return nc


def _get_nc():
    if "nc" not in _CACHE:
        _CACHE["nc"] = _build_nc()
    return _CACHE["nc"]


def make_in_maps(yl, yh1, yh2, g0, g1):
    g0 = np.asarray(g0, np.float32)
    g1 = np.asarray(g1, np.float32)
    mc64 = np.stack([_band_cyclic(g0, 64), _band_cyclic(g1, 64)])
    mc128 = np.stack([_band_cyclic(g0, 128), _band_cyclic(g1, 128)])
    mt2 = np.stack([_band_slab(g0, 68, 128), _band_slab(g1, 68, 128)])
    ident = np.eye(128, dtype=np.float32)
    in_maps = []
    for c in range(8):
        b, h = c // 2, c % 2
        idx68 = np.array([(64 * h - 2 + t) % 128 for t in range(68)])
        ms2 = mc64[:, :, idx68]
        in_maps.append({
            "yl_s": np.ascontiguousarray(yl[b, 0]),
            "yh2_s": np.ascontiguousarray(yh2[b]),
            "yh1_s": np.ascontiguousarray(yh1[b][:, :, idx68, :]),
            "ident": ident,
            "mc64": mc64,
            "ms2": np.ascontiguousarray(ms2),
            "mc128": mc128,
            "mt2": mt2,
        })
    return in_maps


def assemble(results):
    out = np.zeros((4, 1, 256, 256, 256), np.float32)
    for c in range(8):
        b, h = c // 2, c % 2
        fin = results[c]["out"].reshape(256, 128, 256)  # (W, H, D)
        out[b, 0, :, 128 * h:128 * (h + 1), :] = fin.transpose(2, 1, 0)
    return out


def kernel(yl, yh1, yh2, g0, g1):
    from concourse.bass_utils import run_bass_kernel_spmd

    nc = _get_nc()
    in_maps = make_in_maps(yl, yh1, yh2, g0, g1)
    res = run_bass_kernel_spmd(nc, in_maps, list(range(8)))
    return assemble(res.results)

